# revision 15
# baseline (speedup 1.0000x reference)
"""Trainium2 Bass kernel for nn_Conv_89713276879316.

Reference semantics (faithful channel bug): take ONLY the last channel of
image [32, 3, 512, 512], zero-pad by 7, cross-correlate with the 15x15
kernel, broadcast the [32, 1, 512, 512] result to all 3 channels.

Strategy:
  - Host: extract channel 2, zero-pad to [32, 526, 526], cast fp16.
  - Device (per core, 4 images): 15x15 conv as banded matmuls on TensorE in
    fp16 (1 stream pass; ~2e-4 rel err; fp32 PSUM accumulation). For an
    output tile of m rows, 15 matmuls (one per kernel column dx) contract
    the m+14 padded input rows against a banded matrix
    B_dx[r, y] = w[r - y, dx]; dx offsets the moving operand's start
    column; all 15 accumulate into one PSUM bank.
  - Matmuls are issued dx-major over groups of GROUP=4 tiles, so
    consecutive matmuls share their stationary operand (one LDWEIGHTS per
    dx per group instead of one per matmul, freeing the PE's second SBUF
    read port for the fp16 moving stream) and accumulation-group
    boundaries drop to one per group. 2*GROUP = all 8 PSUM banks, so the
    next group's matmuls overlap this group's DVE drains.
  - Cross-image packed tiling: per core the 4 images' 2048 output rows are
    covered by 19 tiles instead of 20 — a tile that would be left with
    <114 rows at an image's bottom edge is packed together with the next
    image's top rows in the same 128 partitions, using a block-diagonal
    band matrix (two band blocks at different partition offsets). This
    removes one full 15-matmul stream.
  - Host: gather 8 shards, broadcast channel dim to 3.

Measured on 8 axon-tunneled trn2 cores (For_i loop-sweep timing, see
bench.loop_sweep): f32r runs ~4.4x slower than fp16 on HW. Progression:
f32r tile-major 283.5us -> fp16+PADW 77us -> dx-major 46.1us.
"""

import sys

import numpy as np

try:
    import concourse.bass as bass
except ImportError:  # pragma: no cover - fallback path inside the container
    sys.path.insert(0, "/opt/trn_rl_repo")
    import concourse.bass as bass

import ml_dtypes
from contextlib import ExitStack

import concourse.tile as tile
from concourse import bacc, mybir
from concourse.bass_utils import run_bass_kernel_spmd

N_CORES = 8
N_IMG = 32
C_IMG = 3
H = W = 512
KS = 15
PAD = KS // 2  # 7
HP = H + 2 * PAD  # 526
PER_CORE = N_IMG // N_CORES  # 4
MTILE = 114  # output rows per matmul tile (contract = MTILE + 14 <= 128)

F32 = mybir.dt.float32

# mode: "f16x2" = fp16 hi/lo 2-pass (~1e-6), "f16" = single pass (~2e-4),
#       "f32r" = single pass float32r (~1e-4 but ~4.4x slower on HW)
import os

MODE = os.environ.get("K_MODE", "f16")
# tunables (env overrides are a dev convenience; defaults are the shipped config)
GROUP = int(os.environ.get("K_GROUP", "4"))
                 # tiles per dx-major group; 2*GROUP <= 8 PSUM banks so the
                 # next group's matmuls overlap this group's PSUM drains
PSUM_BUFS = 8
IMG_BUFS_PER_PLANE = 16
OUT_BUFS = 8
COPY_ENGINE = os.environ.get("K_COPY", "dve")  # "dve" | "act" | "alt"
PADW = True  # pad stationary operands to 128 cols (enables FWL for f16)
PACK = os.environ.get("K_PACK", "1") == "1"  # cross-image 19-tile packing
SORT_PACK = True  # group same-btype tiles together

_MODE_CFG = {
    "f16x2": (mybir.dt.float16, np.float16, 2),
    "f16": (mybir.dt.float16, np.float16, 1),
    "bf16x2": (mybir.dt.bfloat16, ml_dtypes.bfloat16, 2),
    "f32r": (mybir.dt.float32r, np.float32, 1),
    "f32": (mybir.dt.float32, np.float32, 1),
    # fp8 DoubleRow, hi/lo across the two row-slots: out += B.T@hi + B.T@lo
    # in ONE DoubleRow matmul at ~1.44x bf16 throughput; the 0/1 kernel is
    # exact in fp8 and hi+lo quantization error is ~8e-4.
    "f8dr": (mybir.dt.float8e4, ml_dtypes.float8_e4m3, 2),
}

_CACHE = {}


def _build_worklist():
    """Pack the 4 images' 2048 output rows into 19 tiles.

    Each tile is (segs, btype): segs = [(img, y0, m, poff)] gives output
    rows y0..y0+m of image img, its m+14 input rows DMA'd to partitions
    poff..poff+m+14. btype selects the (block-diagonal) band matrix; 0 is
    the standard single-band type. PSUM row of a segment's outputs =
    cumulative output count of earlier segments in the tile.
    """
    tiles = []
    sp_types = []  # (m_a, m_b, poff_b)
    if not PACK:
        for i in range(PER_CORE):
            y = 0
            while y < H:
                m = min(MTILE, H - y)
                tiles.append(([(i, y, m, 0)], 0))
                y += m
        return tiles, sp_types
    i, y = 0, 0
    while i < PER_CORE:
        rem = H - y
        if rem >= MTILE:
            tiles.append(([(i, y, MTILE, 0)], 0))
            y += MTILE
            if y == H:
                i, y = i + 1, 0
        elif i == PER_CORE - 1:
            tiles.append(([(i, y, rem, 0)], 0))
            i, y = i + 1, 0
        else:
            m_a = rem
            poff_b = m_a + KS - 1
            m_b = 128 - poff_b - (KS - 1)  # = 100 - m_a
            sp_types.append((m_a, m_b, poff_b))
            tiles.append(
                ([(i, y, m_a, 0), (i + 1, 0, m_b, poff_b)], len(sp_types))
            )
            i, y = i + 1, m_b
    if SORT_PACK:
        # group same-btype tiles together (dx-major LDWEIGHTS reuse)
        tiles.sort(key=lambda t: t[1] != 0)
    return tiles, sp_types


WORKLIST, SP_TYPES = _build_worklist()
NT = 1 + len(SP_TYPES)  # band types


def _build_nc(repeat=1, mode=None, loop=False):
    """Build the per-core Bass program (identical on all 8 cores).

    repeat > 1 re-runs the whole compute (same inputs/outputs) for
    dispatch-floor-free device timing; with loop=True the repetition is a
    Tile For_i loop instead of unrolling.
    """
    mode = mode or MODE
    mdt, _npdt, passes = _MODE_CFG[mode]
    dr = mode == "f8dr"
    nc = bacc.Bacc("TRN2", target_bir_lowering=False, debug=False)

    mw = 128 if PADW else MTILE  # stationary operand column count
    imgs = [
        nc.dram_tensor(f"img{p}", [PER_CORE, HP, HP], mdt, kind="ExternalInput").ap()
        for p in range(passes)
    ]
    nslot = 2 if dr else 1
    bands = nc.dram_tensor(
        "bands", [128, NT * KS * nslot * mw], mdt, kind="ExternalInput"
    ).ap()
    out = nc.dram_tensor("out", [PER_CORE, H, W], F32, kind="ExternalOutput").ap()

    with tile.TileContext(nc) as tc, ExitStack() as ctx:
        bands_pool = ctx.enter_context(tc.tile_pool(name="bands", bufs=1))
        img_pool = ctx.enter_context(
            tc.tile_pool(name="img", bufs=IMG_BUFS_PER_PLANE * passes)
        )
        psum_pool = ctx.enter_context(
            tc.tile_pool(name="psum", bufs=PSUM_BUFS, space="PSUM")
        )
        out_pool = ctx.enter_context(tc.tile_pool(name="outp", bufs=OUT_BUFS))

        if dr:
            bands_sb = bands_pool.tile([128, NT * KS, 2, mw], mdt)
            nc.sync.dma_start(bands_sb[:], bands[:, :])
            n_mm = KS  # one DoubleRow matmul covers both hi/lo planes
        else:
            bands_sb = bands_pool.tile([128, NT * KS * mw], mdt)
            nc.sync.dma_start(bands_sb[:], bands[:, :])
            n_mm = passes * KS
        cnt = 0

        def emit_out(segs, ps):
            nonlocal cnt
            m_total = sum(m for (_i, _y0, m, _po) in segs)
            ot = out_pool.tile([MTILE, W], F32, name="ot", tag="ot")
            eng = COPY_ENGINE
            if eng == "alt":
                eng = "dve" if cnt % 2 == 0 else "act"
            if eng == "dve":
                nc.vector.tensor_copy(ot[:m_total, :], ps[:m_total, :])
            else:
                nc.scalar.copy(ot[:m_total, :], ps[:m_total, :])
            cnt += 1
            row0 = 0
            for (i, y0, m, _po) in segs:
                nc.sync.dma_start(out[i, y0 : y0 + m, :], ot[row0 : row0 + m, :])
                row0 += m

        def body(_iv=None):
            for g in range(0, len(WORKLIST), GROUP):
                group = WORKLIST[g : g + GROUP]
                units = []  # (segs, btype, r_tile, srcs, ps)
                for (segs, btype) in group:
                    last = segs[-1]
                    r_tile = last[3] + last[2] + KS - 1  # poff + m + 14
                    srcs = []
                    if dr:
                        t = img_pool.tile([128, 2, HP], mdt, name="imgt", tag="img")
                        for p in range(passes):
                            for (i, y0, m, poff) in segs:
                                nc.sync.dma_start(
                                    t[poff : poff + m + KS - 1, p, :],
                                    imgs[p][i, y0 : y0 + m + KS - 1, :],
                                )
                        srcs.append(t)
                    else:
                        for p in range(passes):
                            t = img_pool.tile(
                                [128, HP], mdt, name="imgt", tag=f"img{p}"
                            )
                            for (i, y0, m, poff) in segs:
                                nc.sync.dma_start(
                                    t[poff : poff + m + KS - 1, :],
                                    imgs[p][i, y0 : y0 + m + KS - 1, :],
                                )
                            srcs.append(t)
                    ps = psum_pool.tile([mw, W], F32, name="ps", tag="ps")
                    units.append((segs, btype, r_tile, srcs, ps))

                # dx-major: all tiles of the group share each (pass, dx)
                # stationary -> LDWEIGHTS reuse across consecutive matmuls
                for k in range(n_mm):
                    p, dx = divmod(k, KS)
                    for (segs, btype, r_tile, srcs, ps) in units:
                        mm_cols = mw if PADW else sum(s[2] for s in segs)
                        if dr:
                            nc.tensor.matmul(
                                ps[:mm_cols, :],
                                bands_sb[:r_tile, btype * KS + dx, :, :mm_cols],
                                srcs[0][:r_tile, :, dx : dx + W],
                                start=(k == 0),
                                stop=(k == n_mm - 1),
                                perf_mode=mybir.MatmulPerfMode.DoubleRow,
                            )
                        else:
                            off = (btype * KS + dx) * mw
                            nc.tensor.matmul(
                                ps[:mm_cols, :],
                                bands_sb[:r_tile, off : off + mm_cols],
                                srcs[p][:r_tile, dx : dx + W],
                                start=(k == 0),
                                stop=(k == n_mm - 1),
                            )

                for (segs, _bt, _r, _srcs, ps) in units:
                    emit_out(segs, ps)

        if loop and repeat > 1:
            # unroll 8 bodies per For_i iteration so the ~2us back-edge
            # barrier and lost cross-iteration overlap amortize away
            chunk = 8 if repeat % 8 == 0 else 1
            with tc.For_i(0, repeat // chunk, 1):
                for _u in range(chunk):
                    body()
        else:
            for _rep in range(repeat):
                body()

    nc.compile()
    return nc


def _prep_inputs(image: np.ndarray, kernel: np.ndarray, mode=None):
    """Host-side prep: channel select, pad, hi/lo split, band matrices."""
    mode = mode or MODE
    _mdt, npdt, passes = _MODE_CFG[mode]
    ch = np.ascontiguousarray(image[:, -1, :, :]).astype(np.float32)  # [32,512,512]
    padded = np.zeros((N_IMG, HP, HP), np.float32)
    padded[:, PAD : PAD + H, PAD : PAD + W] = ch
    planes = []
    rem = padded
    for p in range(passes):
        q = rem.astype(npdt)
        planes.append(q)
        if p + 1 < passes:
            rem = rem - q.astype(np.float32)

    w = kernel.astype(np.float32)
    mw = 128 if PADW else MTILE
    bands = np.zeros((128, NT, KS, mw), np.float32)
    for c in range(min(MTILE, mw)):
        bands[c : c + KS, 0, :, c] = w  # B[r, dx, c] = w[r - c, dx]
    for t, (m_a, m_b, poff_b) in enumerate(SP_TYPES, start=1):
        for c in range(m_a):
            bands[c : c + KS, t, :, c] = w
        for j in range(m_b):
            bands[poff_b + j : poff_b + j + KS, t, :, m_a + j] = w
    if mode == "f8dr":
        # duplicate the band into both DoubleRow slots: [128, NT, KS, 2, mw]
        bands = np.broadcast_to(
            bands[:, :, :, None, :], (128, NT, KS, 2, mw)
        )
        bands_c = np.ascontiguousarray(bands).reshape(
            128, NT * KS * 2 * mw
        ).astype(npdt)
    else:
        bands_c = bands.reshape(128, NT * KS * mw).astype(npdt)
    return planes, bands_c


def kernel(image: np.ndarray, kernel: np.ndarray) -> np.ndarray:
    planes, bands_c = _prep_inputs(image, kernel)

    key = ("nc", MODE)
    if key not in _CACHE:
        _CACHE[key] = _build_nc()
    nc = _CACHE[key]

    in_maps = []
    for c in range(N_CORES):
        s = slice(c * PER_CORE, (c + 1) * PER_CORE)
        m = {f"img{p}": planes[p][s] for p in range(len(planes))}
        m["bands"] = bands_c
        in_maps.append(m)

    res = run_bass_kernel_spmd(nc, in_maps, core_ids=list(range(N_CORES)))
    _CACHE["last_results"] = res

    full = np.concatenate([res.results[c]["out"] for c in range(N_CORES)], axis=0)
    out = np.broadcast_to(full[:, None, :, :], (N_IMG, C_IMG, H, W))
    return np.ascontiguousarray(out)


# revision 16
# speedup vs baseline: 1.3434x; 1.3434x over previous
"""Trainium2 Bass kernel for nn_Conv_89713276879316.

Reference semantics (faithful channel bug): take ONLY the last channel of
image [32, 3, 512, 512], zero-pad by 7, cross-correlate with the 15x15
kernel, broadcast the [32, 1, 512, 512] result to all 3 channels.

Strategy:
  - Host: extract channel 2, zero-pad to [32, 526, 526], cast fp16.
  - Device (per core, 4 images): 15x15 conv as banded matmuls on TensorE in
    fp16 (1 stream pass; ~2e-4 rel err; fp32 PSUM accumulation). For an
    output tile of m rows, 15 matmuls (one per kernel column dx) contract
    the m+14 padded input rows against a banded matrix
    B_dx[r, y] = w[r - y, dx]; dx offsets the moving operand's start
    column; all 15 accumulate into one PSUM bank.
  - Matmuls are issued dx-major over groups of GROUP=4 tiles, so
    consecutive matmuls share their stationary operand (one LDWEIGHTS per
    dx per group instead of one per matmul, freeing the PE's second SBUF
    read port for the fp16 moving stream) and accumulation-group
    boundaries drop to one per group. 2*GROUP = all 8 PSUM banks, so the
    next group's matmuls overlap this group's DVE drains.
  - Cross-image packed tiling: per core the 4 images' 2048 output rows are
    covered by 19 tiles instead of 20 — a tile that would be left with
    <114 rows at an image's bottom edge is packed together with the next
    image's top rows in the same 128 partitions, using a block-diagonal
    band matrix (two band blocks at different partition offsets). This
    removes one full 15-matmul stream.
  - Host: gather 8 shards, broadcast channel dim to 3.

Measured on 8 axon-tunneled trn2 cores (For_i loop-sweep timing, see
bench.loop_sweep; steady-state noise +-3us): f32r runs ~4.4x slower than
fp16 on HW (929 ns/MM vs 213). Progression: f32r tile-major 283.5us ->
fp16+PADW tile-major 77us -> fp16 dx-major 19-tile 62-68us (~213 ns/MM,
at the 1 col/cycle @2.4GHz PE stream roofline). MODE="f8dr" (fp8
DoubleRow with hi/lo planes in the two row-slots, W duplicated) is
numerically correct at 8.1e-4 but measured slower (76.8us) - the DR rate
gain doesn't materialize and its 256-col LDWEIGHTS can't FWL.
"""

import sys

import numpy as np

try:
    import concourse.bass as bass
except ImportError:  # pragma: no cover - fallback path inside the container
    sys.path.insert(0, "/opt/trn_rl_repo")
    import concourse.bass as bass

import ml_dtypes
from contextlib import ExitStack

import concourse.tile as tile
from concourse import bacc, mybir
from concourse.bass_utils import run_bass_kernel_spmd

N_CORES = 8
N_IMG = 32
C_IMG = 3
H = W = 512
KS = 15
PAD = KS // 2  # 7
HP = H + 2 * PAD  # 526
PER_CORE = N_IMG // N_CORES  # 4
MTILE = 114  # output rows per matmul tile (contract = MTILE + 14 <= 128)

F32 = mybir.dt.float32

# mode: "f16x2" = fp16 hi/lo 2-pass (~1e-6), "f16" = single pass (~2e-4),
#       "f32r" = single pass float32r (~1e-4 but ~4.4x slower on HW)
import os

MODE = os.environ.get("K_MODE", "f16")
# tunables (env overrides are a dev convenience; defaults are the shipped config)
GROUP = int(os.environ.get("K_GROUP", "4"))
                 # tiles per dx-major group; 2*GROUP <= 8 PSUM banks so the
                 # next group's matmuls overlap this group's PSUM drains
PSUM_BUFS = 8
IMG_BUFS_PER_PLANE = 16
OUT_BUFS = 8
COPY_ENGINE = os.environ.get("K_COPY", "dve")  # "dve" | "act" | "alt"
PADW = True  # pad stationary operands to 128 cols (enables FWL for f16)
PACK = os.environ.get("K_PACK", "1") == "1"  # cross-image 19-tile packing
SORT_PACK = True  # group same-btype tiles together

_MODE_CFG = {
    "f16x2": (mybir.dt.float16, np.float16, 2),
    "f16": (mybir.dt.float16, np.float16, 1),
    "bf16x2": (mybir.dt.bfloat16, ml_dtypes.bfloat16, 2),
    "f32r": (mybir.dt.float32r, np.float32, 1),
    "f32": (mybir.dt.float32, np.float32, 1),
    # fp8 DoubleRow, hi/lo across the two row-slots: out += B.T@hi + B.T@lo
    # in ONE DoubleRow matmul at ~1.44x bf16 throughput; the 0/1 kernel is
    # exact in fp8 and hi+lo quantization error is ~8e-4.
    "f8dr": (mybir.dt.float8e4, ml_dtypes.float8_e4m3, 2),
}

_CACHE = {}


def _build_worklist():
    """Pack the 4 images' 2048 output rows into 19 tiles.

    Each tile is (segs, btype): segs = [(img, y0, m, poff)] gives output
    rows y0..y0+m of image img, its m+14 input rows DMA'd to partitions
    poff..poff+m+14. btype selects the (block-diagonal) band matrix; 0 is
    the standard single-band type. PSUM row of a segment's outputs =
    cumulative output count of earlier segments in the tile.
    """
    tiles = []
    sp_types = []  # (m_a, m_b, poff_b)
    if not PACK:
        for i in range(PER_CORE):
            y = 0
            while y < H:
                m = min(MTILE, H - y)
                tiles.append(([(i, y, m, 0)], 0))
                y += m
        return tiles, sp_types
    i, y = 0, 0
    while i < PER_CORE:
        rem = H - y
        if rem >= MTILE:
            tiles.append(([(i, y, MTILE, 0)], 0))
            y += MTILE
            if y == H:
                i, y = i + 1, 0
        elif i == PER_CORE - 1:
            tiles.append(([(i, y, rem, 0)], 0))
            i, y = i + 1, 0
        else:
            m_a = rem
            poff_b = m_a + KS - 1
            m_b = 128 - poff_b - (KS - 1)  # = 100 - m_a
            sp_types.append((m_a, m_b, poff_b))
            tiles.append(
                ([(i, y, m_a, 0), (i + 1, 0, m_b, poff_b)], len(sp_types))
            )
            i, y = i + 1, m_b
    if SORT_PACK:
        # group same-btype tiles together (dx-major LDWEIGHTS reuse)
        tiles.sort(key=lambda t: t[1] != 0)
    return tiles, sp_types


WORKLIST, SP_TYPES = _build_worklist()
NT = 1 + len(SP_TYPES)  # band types


def _build_nc(repeat=1, mode=None, loop=False):
    """Build the per-core Bass program (identical on all 8 cores).

    repeat > 1 re-runs the whole compute (same inputs/outputs) for
    dispatch-floor-free device timing; with loop=True the repetition is a
    Tile For_i loop instead of unrolling.
    """
    mode = mode or MODE
    mdt, _npdt, passes = _MODE_CFG[mode]
    dr = mode == "f8dr"
    nc = bacc.Bacc("TRN2", target_bir_lowering=False, debug=False)

    mw = 128 if PADW else MTILE  # stationary operand column count
    imgs = [
        nc.dram_tensor(f"img{p}", [PER_CORE, HP, HP], mdt, kind="ExternalInput").ap()
        for p in range(passes)
    ]
    nslot = 2 if dr else 1
    bands = nc.dram_tensor(
        "bands", [128, NT * KS * nslot * mw], mdt, kind="ExternalInput"
    ).ap()
    out = nc.dram_tensor("out", [PER_CORE, H, W], F32, kind="ExternalOutput").ap()

    with tile.TileContext(nc) as tc, ExitStack() as ctx:
        bands_pool = ctx.enter_context(tc.tile_pool(name="bands", bufs=1))
        img_pool = ctx.enter_context(
            tc.tile_pool(name="img", bufs=IMG_BUFS_PER_PLANE * passes)
        )
        psum_pool = ctx.enter_context(
            tc.tile_pool(name="psum", bufs=PSUM_BUFS, space="PSUM")
        )
        out_pool = ctx.enter_context(tc.tile_pool(name="outp", bufs=OUT_BUFS))

        if dr:
            bands_sb = bands_pool.tile([128, NT * KS, 2, mw], mdt)
            nc.sync.dma_start(bands_sb[:], bands[:, :])
            n_mm = KS  # one DoubleRow matmul covers both hi/lo planes
        else:
            bands_sb = bands_pool.tile([128, NT * KS * mw], mdt)
            nc.sync.dma_start(bands_sb[:], bands[:, :])
            n_mm = passes * KS
        cnt = 0

        def emit_out(segs, ps):
            nonlocal cnt
            m_total = sum(m for (_i, _y0, m, _po) in segs)
            ot = out_pool.tile([MTILE, W], F32, name="ot", tag="ot")
            eng = COPY_ENGINE
            if eng == "alt":
                eng = "dve" if cnt % 2 == 0 else "act"
            if eng == "dve":
                nc.vector.tensor_copy(ot[:m_total, :], ps[:m_total, :])
            else:
                nc.scalar.copy(ot[:m_total, :], ps[:m_total, :])
            cnt += 1
            row0 = 0
            for (i, y0, m, _po) in segs:
                nc.sync.dma_start(out[i, y0 : y0 + m, :], ot[row0 : row0 + m, :])
                row0 += m

        def body(_iv=None):
            for g in range(0, len(WORKLIST), GROUP):
                group = WORKLIST[g : g + GROUP]
                units = []  # (segs, btype, r_tile, srcs, ps)
                for (segs, btype) in group:
                    last = segs[-1]
                    r_tile = last[3] + last[2] + KS - 1  # poff + m + 14
                    srcs = []
                    if dr:
                        t = img_pool.tile([128, 2, HP], mdt, name="imgt", tag="img")
                        for p in range(passes):
                            for (i, y0, m, poff) in segs:
                                nc.sync.dma_start(
                                    t[poff : poff + m + KS - 1, p, :],
                                    imgs[p][i, y0 : y0 + m + KS - 1, :],
                                )
                        srcs.append(t)
                    else:
                        for p in range(passes):
                            t = img_pool.tile(
                                [128, HP], mdt, name="imgt", tag=f"img{p}"
                            )
                            for (i, y0, m, poff) in segs:
                                nc.sync.dma_start(
                                    t[poff : poff + m + KS - 1, :],
                                    imgs[p][i, y0 : y0 + m + KS - 1, :],
                                )
                            srcs.append(t)
                    ps = psum_pool.tile([mw, W], F32, name="ps", tag="ps")
                    units.append((segs, btype, r_tile, srcs, ps))

                # dx-major: all tiles of the group share each (pass, dx)
                # stationary -> LDWEIGHTS reuse across consecutive matmuls
                for k in range(n_mm):
                    p, dx = divmod(k, KS)
                    for (segs, btype, r_tile, srcs, ps) in units:
                        mm_cols = mw if PADW else sum(s[2] for s in segs)
                        if dr:
                            nc.tensor.matmul(
                                ps[:mm_cols, :],
                                bands_sb[:r_tile, btype * KS + dx, :, :mm_cols],
                                srcs[0][:r_tile, :, dx : dx + W],
                                start=(k == 0),
                                stop=(k == n_mm - 1),
                                perf_mode=mybir.MatmulPerfMode.DoubleRow,
                            )
                        else:
                            off = (btype * KS + dx) * mw
                            nc.tensor.matmul(
                                ps[:mm_cols, :],
                                bands_sb[:r_tile, off : off + mm_cols],
                                srcs[p][:r_tile, dx : dx + W],
                                start=(k == 0),
                                stop=(k == n_mm - 1),
                            )

                for (segs, _bt, _r, _srcs, ps) in units:
                    emit_out(segs, ps)

        if loop and repeat > 1:
            # unroll 8 bodies per For_i iteration so the ~2us back-edge
            # barrier and lost cross-iteration overlap amortize away
            chunk = 8 if repeat % 8 == 0 else 1
            with tc.For_i(0, repeat // chunk, 1):
                for _u in range(chunk):
                    body()
        else:
            for _rep in range(repeat):
                body()

    nc.compile()
    return nc


def _prep_inputs(image: np.ndarray, kernel: np.ndarray, mode=None):
    """Host-side prep: channel select, pad, hi/lo split, band matrices."""
    mode = mode or MODE
    _mdt, npdt, passes = _MODE_CFG[mode]
    ch = np.ascontiguousarray(image[:, -1, :, :]).astype(np.float32)  # [32,512,512]
    padded = np.zeros((N_IMG, HP, HP), np.float32)
    padded[:, PAD : PAD + H, PAD : PAD + W] = ch
    planes = []
    rem = padded
    for p in range(passes):
        q = rem.astype(npdt)
        planes.append(q)
        if p + 1 < passes:
            rem = rem - q.astype(np.float32)

    w = kernel.astype(np.float32)
    mw = 128 if PADW else MTILE
    bands = np.zeros((128, NT, KS, mw), np.float32)
    for c in range(min(MTILE, mw)):
        bands[c : c + KS, 0, :, c] = w  # B[r, dx, c] = w[r - c, dx]
    for t, (m_a, m_b, poff_b) in enumerate(SP_TYPES, start=1):
        for c in range(m_a):
            bands[c : c + KS, t, :, c] = w
        for j in range(m_b):
            bands[poff_b + j : poff_b + j + KS, t, :, m_a + j] = w
    if mode == "f8dr":
        # duplicate the band into both DoubleRow slots: [128, NT, KS, 2, mw]
        bands = np.broadcast_to(
            bands[:, :, :, None, :], (128, NT, KS, 2, mw)
        )
        bands_c = np.ascontiguousarray(bands).reshape(
            128, NT * KS * 2 * mw
        ).astype(npdt)
    else:
        bands_c = bands.reshape(128, NT * KS * mw).astype(npdt)
    return planes, bands_c


def kernel(image: np.ndarray, kernel: np.ndarray) -> np.ndarray:
    planes, bands_c = _prep_inputs(image, kernel)

    key = ("nc", MODE)
    if key not in _CACHE:
        _CACHE[key] = _build_nc()
    nc = _CACHE[key]

    in_maps = []
    for c in range(N_CORES):
        s = slice(c * PER_CORE, (c + 1) * PER_CORE)
        m = {f"img{p}": planes[p][s] for p in range(len(planes))}
        m["bands"] = bands_c
        in_maps.append(m)

    res = run_bass_kernel_spmd(nc, in_maps, core_ids=list(range(N_CORES)))
    _CACHE["last_results"] = res

    full = np.concatenate([res.results[c]["out"] for c in range(N_CORES)], axis=0)
    out = np.broadcast_to(full[:, None, :, :], (N_IMG, C_IMG, H, W))
    return np.ascontiguousarray(out)
